# revision 33
# baseline (speedup 1.0000x reference)
"""Trainium2 Bass kernel for windowed (sparse) attention.

Module: LayerNorm -> overlapping 8x8 spatial windows (stride 6) over a
[2,2,128,128,256] image -> per-window 8-head attention over L=128 tokens
(t*8*8) -> output projection -> overlap-add with count normalization.

Strategy: 882 independent windows sharded over 8 cores (112 each, padded
to 896).  Host does LayerNorm + im2win gather + overlap-add scatter
(pure data movement / pointwise prep); all matmul compute (QKV,
attention, Wo) runs on device.

v4: windows are processed in PAIRS so every weight-stationary matmul
streams N=256 (qk projections, softmax-denominator matmuls), halving
the per-window LDWEIGHTS tax (~107ns each, the serial floor of this
runtime).  The xn transposes for pair p+1 are issued at the START of
pair p's body (software pipelining) so the PE never idles on the
transpose-evac chain.  PSUM is laid out so the attention-score region
written by pair p+1 only overlaps regions whose readers completed early
in pair p:
  banks 0-3 (one [128,4,4,128] f32 tile): S^T row-tiled by head group
    (bank hh), col block wi*2+mh; O/Wo outputs reuse cols after exp
  banks 4-5: q|k pair projections [128,4,256]
  bank 6:    v-pair / D-pair (sequential reuse, one [128,2,256] tag)
  bank 7:    PE-transpose output (fp16, double-buffered)

Device dataflow per pair (matmuls fp16 in / fp32 psum):
  xn[128,2,256] --PE-transpose--> xnT[128,kc,wi,128]
  qT/kT = W.T-stationary matmuls N=256  (heads land stacked on
          partitions: head h=mh*4+hh at partitions 32hh of chunk mh)
  v     = xnT-stationary matmuls -> V[l,mid] per window
  S^T_h = row-tiled matmul tile_position=(32hh,0), lhsT=kT_h[32,128],
          rhs=qT_h[32,128] read in place from qks (no rearrange)
  E^T   = exp(S^T/sqrt(32)) one ACT op per window -> es[128,4,2,256]
  D_h   = ones.T @ E^T_h col-packed N=256 over the pair
  Dinv  = reciprocal_approx(D-pair) (pre-broadcast layout)
  OT_u  = V_h-stationary @ E^T_h col-packed; OT = OT_u * Dinv
  ZT    = WoT-stationary @ OT -> fp16 -> one DMA out per window
"""

import functools
import math
from contextlib import ExitStack

import numpy as np

import concourse.bacc as bacc
import concourse.bass as bass
import concourse.mybir as mybir
import concourse.tile as tile
from concourse.bass_utils import run_bass_kernel_spmd

# Problem constants (hardcoded per contract - kernel.py is self-contained).
B, T, H, W, C = 2, 2, 128, 128, 256
MID, HEADS = 256, 8
HD = MID // HEADS          # 32
PATCH, STEP = 8, 6         # window size / stride
NHW = 21                   # windows per axis: starts 0,6,...,120
NWIN = NHW * NHW * B       # 882 flat windows (n outer, b inner)
L = T * PATCH * PATCH      # 128 tokens per window
NCORES = 8
NW = 112                   # windows per core after padding to 896
NPAIR = NW // 2
EPS = 1e-6
F32, F16 = mybir.dt.float32, mybir.dt.float16
AF = mybir.ActivationFunctionType
ALU = mybir.AluOpType


def _build_program(nw: int):
    nc = bacc.Bacc(
        "TRN2",
        target_bir_lowering=False,
        debug=False,
        enable_asserts=False,
        num_devices=NCORES,
    )
    xw = nc.dram_tensor("xw", [nw * 128, 256], F16, kind="ExternalInput").ap()
    wq = nc.dram_tensor("wq", [256, 256], F16, kind="ExternalInput").ap()
    wk = nc.dram_tensor("wk", [256, 256], F16, kind="ExternalInput").ap()
    wv = nc.dram_tensor("wv", [256, 256], F16, kind="ExternalInput").ap()
    wo = nc.dram_tensor("wo", [256, 256], F16, kind="ExternalInput").ap()
    ones1 = nc.dram_tensor("ones1", [128, 32], F16, kind="ExternalInput").ap()
    ident = nc.dram_tensor("ident", [128, 128], F16, kind="ExternalInput").ap()
    zt = nc.dram_tensor("zt", [nw * 128, 256], F16, kind="ExternalOutput").ap()

    inv_sqrt_hd = 1.0 / math.sqrt(HD)
    npair = nw // 2

    with tile.TileContext(nc) as tc, ExitStack() as ctx:
        pw = ctx.enter_context(tc.tile_pool(name="wts", bufs=1))
        # Persistent weight tiles: chunk kc holds input-dim rows kc*128..+128.
        wq_s = [pw.tile([128, 256], F16, tag=f"wq{i}", name=f"wq{i}") for i in range(2)]
        wk_s = [pw.tile([128, 256], F16, tag=f"wk{i}", name=f"wk{i}") for i in range(2)]
        wv_s = [pw.tile([128, 256], F16, tag=f"wv{i}", name=f"wv{i}") for i in range(2)]
        wo_s = [pw.tile([128, 256], F16, tag=f"wo{i}", name=f"wo{i}") for i in range(2)]
        for i in range(2):
            nc.sync.dma_start(wq_s[i][:], wq[i * 128:(i + 1) * 128, :])
            nc.sync.dma_start(wk_s[i][:], wk[i * 128:(i + 1) * 128, :])
            nc.sync.dma_start(wv_s[i][:], wv[i * 128:(i + 1) * 128, :])
            nc.sync.dma_start(wo_s[i][:], wo[i * 128:(i + 1) * 128, :])
        ones_s = pw.tile([128, 32], F16, tag="ones1")
        nc.sync.dma_start(ones_s[:], ones1[:])
        id_s = pw.tile([128, 128], F16, tag="ident")
        nc.sync.dma_start(id_s[:], ident[:])

        # SBUF pools
        px = ctx.enter_context(tc.tile_pool(name="px", bufs=2))
        pxnt = ctx.enter_context(tc.tile_pool(name="pxnt", bufs=2))
        pqks = ctx.enter_context(tc.tile_pool(name="pqks", bufs=2))
        pvs = ctx.enter_context(tc.tile_pool(name="pvs", bufs=2))
        pes = ctx.enter_context(tc.tile_pool(name="pes", bufs=2))
        pdbs = ctx.enter_context(tc.tile_pool(name="pdbs", bufs=2))
        pos = ctx.enter_context(tc.tile_pool(name="pos", bufs=2))
        pzs = ctx.enter_context(tc.tile_pool(name="pzs", bufs=2))
        # PSUM pools: sp 4 banks + qk 2 + vd 1 + tr 1 = 8 banks.
        psp = ctx.enter_context(tc.tile_pool(name="psp", bufs=1, space="PSUM"))
        pqk = ctx.enter_context(tc.tile_pool(name="pqk", bufs=1, space="PSUM"))
        pvd = ctx.enter_context(tc.tile_pool(name="pvd", bufs=1, space="PSUM"))
        ptr = ctx.enter_context(tc.tile_pool(name="ptr", bufs=1, space="PSUM"))

        def load_tr(p):
            """Load pair p's xn and transpose it on the PE -> xnt2.

            Returns (xnt2, trop): trop is the shared transpose/op(w0) PSUM
            bank -- transposes use its upper half (bitcast f16), the body
            that issues this call uses the lower half for O(w0) output.
            """
            xn2 = px.tile([128, 2, 256], F16, tag="x")
            for wi in range(2):
                w = 2 * p + wi
                nc.sync.dma_start(xn2[:, wi, :], xw[w * 128:(w + 1) * 128, :])
            trop = ptr.tile([128, 4, 128], F32, tag="trop")
            trp = trop[:, 2:4, :].bitcast(F16)   # [128, 2, 256] f16 view
            for kc in range(2):
                for wi in range(2):
                    nc.tensor.transpose(
                        trp[:, kc, wi * 128:(wi + 1) * 128],
                        xn2[:, wi, kc * 128:(kc + 1) * 128], id_s[:],
                    )
            xnt2 = pxnt.tile([128, 2, 2, 128], F16, tag="xnt")  # (kc, wi, l)
            nc.vector.tensor_copy(xnt2[:], trp[:])
            return xnt2, trop

        xnt_cur, trop_cur = load_tr(0)

        for p in range(npair):
            w0 = 2 * p
            xnt2 = xnt_cur
            # ---- prologue: load + transpose for pair p+1 ----
            if p + 1 < npair:
                xnt_cur, trop = load_tr(p + 1)
            else:
                trop = ptr.tile([128, 4, 128], F32, tag="trop")

            # ---- q/k projections, N=256 over the pair ----
            # qkp block b: b=0,1 -> q chunk mh=b; b=2,3 -> k chunk mh=b-2.
            qkp = pqk.tile([128, 4, 256], F32, tag="qk")
            for isk, ws in ((0, wq_s), (1, wk_s)):
                for mh in range(2):
                    for kc in range(2):
                        nc.tensor.matmul(
                            qkp[:, isk * 2 + mh, :],
                            lhsT=ws[kc][:, mh * 128:(mh + 1) * 128],
                            rhs=xnt2[:, kc, :, :],
                            start=(mh == 0 and kc == 0),
                            stop=(mh == 1 and kc == 1),
                            skip_group_check=True,
                        )
            qks = pqks.tile([128, 4, 256], F16, tag="qks")  # (b, wi*128+l)
            nc.scalar.activation(qks[:], qkp[:], AF.Copy)

            # ---- v projections (xnT-stationary, N=256 per matmul) ----
            vdp = pvd.tile([128, 2, 256], F32, tag="vd")
            for wi in range(2):
                for kc in range(2):
                    nc.tensor.matmul(
                        vdp[:, wi, :],
                        lhsT=xnt2[:, kc, wi, :], rhs=wv_s[kc][:],
                        start=(kc == 0), stop=(kc == 1),
                        skip_group_check=True,
                    )
            vs2 = pvs.tile([128, 2, 256], F16, tag="vs")
            nc.vector.tensor_copy(vs2[:], vdp[:])

            # ---- S^T row-tiled in place; head h=mh*4+hh at partitions 32hh
            # of chunk mh.  sp block layout: [:, hh, mh*2+wi, :] ----
            sp = psp.tile([128, 4, 4, 128], F32, tag="sp")
            for mh in range(2):
                for wi in range(2):
                    for hh in (0, 1, 2, 3):  # contended banks (2,3) last
                        nc.tensor.matmul(
                            sp[:, hh, mh * 2 + wi, :],
                            lhsT=qks[32 * hh:32 * hh + 32, 2 + mh, wi * 128:(wi + 1) * 128],
                            rhs=qks[32 * hh:32 * hh + 32, mh, wi * 128:(wi + 1) * 128],
                            start=True, stop=True,
                            tile_position=(32 * hh, 0),
                            skip_group_check=True,
                        )
            # E^T = exp(S^T/sqrt(hd)); split by mh so D r=0 can start while
            # the second half still runs on ACT.
            es2 = pes.tile([128, 4, 2, 256], F16, tag="es")
            for mh in range(2):
                nc.scalar.activation(
                    es2[:, :, mh, :], sp[:, :, mh * 2:mh * 2 + 2, :],
                    AF.Exp, scale=inv_sqrt_hd)

            # ---- denominators D, pair-batched N=256 ----
            # dp[32j+d, r, wi*128+l] = D_{4r+j}(wi)[l]  (bank 6 reuse)
            dp = pvd.tile([128, 2, 256], F32, tag="vd")
            for h in range(HEADS):
                r, j = h // 4, h % 4
                nc.tensor.matmul(
                    dp[32 * j:32 * j + 32, r, :],
                    lhsT=ones_s[:], rhs=es2[:, j, r, :],
                    start=(r == 0), stop=(r == 1), tile_position=(0, 32 * j),
                    skip_group_check=True,
                )

            # ---- attention output + out-proj per window, interleaved so the
            # DVE chain (dbs -> os) overlaps the other window's matmuls ----
            # op(w0) -> spare half of the transpose bank; op(w1) -> sp bank2
            # cols 0:256; zp -> sp bank3 (all overwritten only after their
            # readers from the PREVIOUS pair completed mid-pair).
            sp2 = psp.tile([128, 4, 4, 128], F32, tag="sp")
            dbs = pdbs.tile([128, 2, 256], F32, tag="dbs")  # (r, wi*128+l)
            os2 = pos.tile([128, 2, 2, 128], F16, tag="os")
            for wi in range(2):
                nc.vector.reciprocal_approx_fast(
                    out=dbs[:, :, wi * 128:(wi + 1) * 128],
                    in_=dp[:, :, wi * 128:(wi + 1) * 128])
                for h in range(HEADS):
                    r, j = h // 4, h % 4
                    out_sl = (trop[32 * j:32 * j + 32, r, :] if wi == 0 else
                              sp2[32 * j:32 * j + 32, 2, r, :])
                    nc.tensor.matmul(
                        out_sl,
                        lhsT=vs2[:, wi, 32 * h:32 * h + 32],
                        rhs=es2[:, j, r, wi * 128:(wi + 1) * 128],
                        start=(r == 0), stop=(r == 1), tile_position=(0, 32 * j),
                        skip_group_check=True,
                    )
                # fused normalize: os2[p, wi, r, l] = op * 1/D
                nc.vector.scalar_tensor_tensor(
                    out=os2[:, wi, :, :],
                    in0=(trop[:, 0:2, :] if wi == 0 else sp2[:, 2, 0:2, :]),
                    scalar=1.0, in1=dbs[:, :, wi * 128:(wi + 1) * 128],
                    op0=ALU.mult, op1=ALU.mult,
                )
                for coh in range(2):
                    for kc in range(2):
                        nc.tensor.matmul(
                            sp2[:, 3, 2 * wi + coh, :],
                            lhsT=wo_s[kc][:, coh * 128:(coh + 1) * 128],
                            rhs=os2[:, wi, kc, :],
                            start=(coh == 0 and kc == 0),
                            stop=(coh == 1 and kc == 1),
                            skip_group_check=True,
                        )
            # one fp16 evac for both windows, then one store per window.
            zs = pzs.tile([128, 2, 2, 128], F16, tag="zs")  # (wi, coh, l)
            nc.scalar.activation(zs[:], sp2[:, 3, :, :], AF.Copy)
            for wi in range(2):
                w = w0 + wi
                nc.sync.dma_start(zt[w * 128:(w + 1) * 128, :], zs[:, wi, :, :])
    nc.compile()
    return nc


@functools.lru_cache(maxsize=2)
def _get_program(nw: int):
    return _build_program(nw)


def _im2win(x: np.ndarray) -> np.ndarray:
    """[B,T,H,W,C] -> [882,128,256] windows, flat order f = i_n*B + i_b."""
    s = x.strides
    xs = np.lib.stride_tricks.as_strided(
        x,
        shape=(B, T, NHW, PATCH, NHW, PATCH, C),
        strides=(s[0], s[1], STEP * s[2], s[2], STEP * s[3], s[3], s[4]),
    )
    w = xs.transpose(2, 4, 0, 1, 3, 5, 6)  # [iH,iW,b,t,p,q,c]
    return np.ascontiguousarray(w.reshape(NHW * NHW * B, L, C))


def _overlap_add(zwin: np.ndarray, bo: np.ndarray) -> np.ndarray:
    """[882,128,256] window outputs -> [B,T,H,W,C] with count-normalize + bo.

    Reproduces the reference's (n*b)->(b,n) flat-order reinterpretation.
    """
    th = np.arange(NHW) * STEP
    z = zwin.reshape(B, NHW, NHW, T, PATCH, PATCH, MID)  # [b,iH,iW,t,p,q,c]
    acc = np.zeros((B, T, H, W, MID), np.float32)
    count = np.zeros((H, W), np.float32)
    for p in range(PATCH):
        rid = (th + p)[:, None]
        for q in range(PATCH):
            cid = (th + q)[None, :]
            acc[:, :, rid, cid, :] += z[:, :, :, :, p, q, :].transpose(0, 3, 1, 2, 4)
            count[rid, cid] += 1.0
    out = acc / count[None, None, :, :, None] + bo[None, None, None, None, :]
    return out


LAST_RESULT = None


def kernel(x, ln_g, ln_b, Wq, Wk, Wv, Wo, bo):
    x = np.asarray(x, np.float32)
    ln_g = np.asarray(ln_g, np.float32)
    ln_b = np.asarray(ln_b, np.float32)
    assert np.allclose(ln_b, 0.0), "kernel folds ln_g into weights; ln_b must be 0"
    # Fold LN gamma into the input side of Wq/Wk/Wv.
    wq_t = np.ascontiguousarray((np.asarray(Wq, np.float32) * ln_g).T.astype(np.float16))
    wk_t = np.ascontiguousarray((np.asarray(Wk, np.float32) * ln_g).T.astype(np.float16))
    wv_t = np.ascontiguousarray((np.asarray(Wv, np.float32) * ln_g).T.astype(np.float16))
    wo_t = np.ascontiguousarray(np.asarray(Wo, np.float32).T.astype(np.float16))
    ones1 = np.ones((128, 32), np.float16)
    ident = np.eye(128, dtype=np.float16)

    # LayerNorm on the host (pure data prep; gamma folded into W, beta=0),
    # then im2win.  fp16 halves the per-window DMA load.
    mu = x.mean(-1, keepdims=True)
    var = x.var(-1, keepdims=True)
    xnh = ((x - mu) / np.sqrt(var + EPS)).astype(np.float16)
    win = _im2win(xnh)                            # [882, 128, 256] fp16
    pad = NCORES * NW - NWIN                      # 14
    winp = np.concatenate([win, np.zeros((pad, L, C), np.float16)], 0)
    shards = winp.reshape(NCORES, NW * L, C)

    nc = _get_program(NW)
    trace = bool(int(__import__("os").environ.get("KERNEL_TRACE", "0")))
    in_maps = []
    for i in range(NCORES):
        in_maps.append({
            "xw": np.ascontiguousarray(shards[i]),
            "wq": wq_t, "wk": wk_t, "wv": wv_t, "wo": wo_t,
            "ones1": ones1, "ident": ident,
        })
    res = run_bass_kernel_spmd(nc, in_maps, core_ids=list(range(NCORES)),
                               trace=trace)
    global LAST_RESULT
    LAST_RESULT = res
    # zt rows w*128+p, cols coh*128+l hold Z_w[l, coh*128+p]
    zts = [res.results[i]["zt"].reshape(NW, 128, 2, 128) for i in range(NCORES)]
    zall = np.concatenate(zts, 0).astype(np.float32)   # [896, p, coh, l]
    zwin = zall.transpose(0, 3, 2, 1).reshape(NCORES * NW, L, MID)[:NWIN]
    return _overlap_add(np.ascontiguousarray(zwin), np.asarray(bo, np.float32))


# revision 34
# speedup vs baseline: 1.1310x; 1.1310x over previous
"""Trainium2 Bass kernel for windowed (sparse) attention.

Module: LayerNorm -> overlapping 8x8 spatial windows (stride 6) over a
[2,2,128,128,256] image -> per-window 8-head attention over L=128 tokens
(t*8*8) -> output projection -> overlap-add with count normalization.

Strategy: 882 independent windows sharded over 8 cores (112 each, padded
to 896).  Host does LayerNorm + im2win gather + overlap-add scatter
(pure data movement / pointwise prep); all matmul compute (QKV,
attention, Wo) runs on device.

v4: windows are processed in PAIRS so every weight-stationary matmul
streams N=256 (qk projections, softmax-denominator matmuls), halving
the per-window LDWEIGHTS tax (~107ns each, the serial floor of this
runtime).  The xn transposes for pair p+1 are issued at the START of
pair p's body (software pipelining) so the PE never idles on the
transpose-evac chain.  PSUM is laid out so the attention-score region
written by pair p+1 only overlaps regions whose readers completed early
in pair p:
  banks 0-3 (one [128,4,4,128] f32 tile): S^T row-tiled by head group
    (bank hh), col block wi*2+mh; O/Wo outputs reuse cols after exp
  banks 4-5: q|k pair projections [128,4,256]
  bank 6:    v-pair / D-pair (sequential reuse, one [128,2,256] tag)
  bank 7:    PE-transpose output (fp16, double-buffered)

Device dataflow per pair (matmuls fp16 in / fp32 psum):
  xn[128,2,256] --PE-transpose--> xnT[128,kc,wi,128]
  qT/kT = W.T-stationary matmuls N=256  (heads land stacked on
          partitions: head h=mh*4+hh at partitions 32hh of chunk mh)
  v     = xnT-stationary matmuls -> V[l,mid] per window
  S^T_h = row-tiled matmul tile_position=(32hh,0), lhsT=kT_h[32,128],
          rhs=qT_h[32,128] read in place from qks (no rearrange)
  E^T   = exp(S^T/sqrt(32)) one ACT op per window -> es[128,4,2,256]
  D_h   = ones.T @ E^T_h col-packed N=256 over the pair
  Dinv  = reciprocal_approx(D-pair) (pre-broadcast layout)
  OT_u  = V_h-stationary @ E^T_h col-packed; OT = OT_u * Dinv
  ZT    = WoT-stationary @ OT -> fp16 -> one DMA out per window
"""

import functools
import math
from contextlib import ExitStack

import numpy as np

import concourse.bacc as bacc
import concourse.bass as bass
import concourse.mybir as mybir
import concourse.tile as tile
from concourse.bass_utils import run_bass_kernel_spmd

# Problem constants (hardcoded per contract - kernel.py is self-contained).
B, T, H, W, C = 2, 2, 128, 128, 256
MID, HEADS = 256, 8
HD = MID // HEADS          # 32
PATCH, STEP = 8, 6         # window size / stride
NHW = 21                   # windows per axis: starts 0,6,...,120
NWIN = NHW * NHW * B       # 882 flat windows (n outer, b inner)
L = T * PATCH * PATCH      # 128 tokens per window
NCORES = 8
NW = 112                   # windows per core after padding to 896
NPAIR = NW // 2
EPS = 1e-6
F32, F16 = mybir.dt.float32, mybir.dt.float16
AF = mybir.ActivationFunctionType
ALU = mybir.AluOpType


def _build_program(nw: int):
    nc = bacc.Bacc(
        "TRN2",
        target_bir_lowering=False,
        debug=False,
        enable_asserts=False,
        num_devices=NCORES,
    )
    xw = nc.dram_tensor("xw", [nw * 128, 256], F16, kind="ExternalInput").ap()
    wq = nc.dram_tensor("wq", [256, 256], F16, kind="ExternalInput").ap()
    wk = nc.dram_tensor("wk", [256, 256], F16, kind="ExternalInput").ap()
    wv = nc.dram_tensor("wv", [256, 256], F16, kind="ExternalInput").ap()
    wo = nc.dram_tensor("wo", [256, 256], F16, kind="ExternalInput").ap()
    ones1 = nc.dram_tensor("ones1", [128, 32], F16, kind="ExternalInput").ap()
    ident = nc.dram_tensor("ident", [128, 128], F16, kind="ExternalInput").ap()
    zt = nc.dram_tensor("zt", [nw * 128, 256], F16, kind="ExternalOutput").ap()

    inv_sqrt_hd = 1.0 / math.sqrt(HD)
    npair = nw // 2

    with tile.TileContext(nc) as tc, ExitStack() as ctx:
        pw = ctx.enter_context(tc.tile_pool(name="wts", bufs=1))
        # Persistent weight tiles: chunk kc holds input-dim rows kc*128..+128.
        wq_s = [pw.tile([128, 256], F16, tag=f"wq{i}", name=f"wq{i}") for i in range(2)]
        wk_s = [pw.tile([128, 256], F16, tag=f"wk{i}", name=f"wk{i}") for i in range(2)]
        wv_s = [pw.tile([128, 256], F16, tag=f"wv{i}", name=f"wv{i}") for i in range(2)]
        wo_s = [pw.tile([128, 256], F16, tag=f"wo{i}", name=f"wo{i}") for i in range(2)]
        for i in range(2):
            nc.sync.dma_start(wq_s[i][:], wq[i * 128:(i + 1) * 128, :])
            nc.sync.dma_start(wk_s[i][:], wk[i * 128:(i + 1) * 128, :])
            nc.sync.dma_start(wv_s[i][:], wv[i * 128:(i + 1) * 128, :])
            nc.sync.dma_start(wo_s[i][:], wo[i * 128:(i + 1) * 128, :])
        ones_s = pw.tile([128, 32], F16, tag="ones1")
        nc.sync.dma_start(ones_s[:], ones1[:])
        id_s = pw.tile([128, 128], F16, tag="ident")
        nc.sync.dma_start(id_s[:], ident[:])

        # SBUF pools
        px = ctx.enter_context(tc.tile_pool(name="px", bufs=2))
        pxnt = ctx.enter_context(tc.tile_pool(name="pxnt", bufs=2))
        pqks = ctx.enter_context(tc.tile_pool(name="pqks", bufs=2))
        pvs = ctx.enter_context(tc.tile_pool(name="pvs", bufs=2))
        pes = ctx.enter_context(tc.tile_pool(name="pes", bufs=2))
        pdbs = ctx.enter_context(tc.tile_pool(name="pdbs", bufs=2))
        pos = ctx.enter_context(tc.tile_pool(name="pos", bufs=2))
        pzs = ctx.enter_context(tc.tile_pool(name="pzs", bufs=2))
        # PSUM pools: sp 4 banks + qk 2 + vd 1 + tr 1 = 8 banks.
        psp = ctx.enter_context(tc.tile_pool(name="psp", bufs=1, space="PSUM"))
        pqk = ctx.enter_context(tc.tile_pool(name="pqk", bufs=1, space="PSUM"))
        pvd = ctx.enter_context(tc.tile_pool(name="pvd", bufs=1, space="PSUM"))
        ptr = ctx.enter_context(tc.tile_pool(name="ptr", bufs=1, space="PSUM"))

        def emit_loads(p):
            """DMA pair p's xn windows into SBUF."""
            xn2 = px.tile([128, 2, 256], F16, tag="x")
            for wi in range(2):
                w = 2 * p + wi
                nc.sync.dma_start(xn2[:, wi, :], xw[w * 128:(w + 1) * 128, :])
            return xn2

        def emit_tr(xn2):
            """PE-transpose pair p's xn -> xnt2 (kc, wi, l)."""
            trp = ptr.tile([128, 2, 256], F16, tag="tr")
            for kc in range(2):
                for wi in range(2):
                    nc.tensor.transpose(
                        trp[:, kc, wi * 128:(wi + 1) * 128],
                        xn2[:, wi, kc * 128:(kc + 1) * 128], id_s[:],
                    )
            xnt2 = pxnt.tile([128, 2, 2, 128], F16, tag="xnt")
            nc.vector.tensor_copy(xnt2[:], trp[:])
            return xnt2

        def emit_qkv(xnt2):
            """q/k (weight-stationary, N=256) and v projections + evacs."""
            qkp = pqk.tile([128, 4, 256], F32, tag="qk")
            for isk, ws in ((0, wq_s), (1, wk_s)):
                for mh in range(2):
                    for kc in range(2):
                        nc.tensor.matmul(
                            qkp[:, isk * 2 + mh, :],
                            lhsT=ws[kc][:, mh * 128:(mh + 1) * 128],
                            rhs=xnt2[:, kc, :, :],
                            start=(mh == 0 and kc == 0),
                            stop=(mh == 1 and kc == 1),
                            skip_group_check=True,
                        )
            qks = pqks.tile([128, 4, 256], F16, tag="qks")  # (b, wi*128+l)
            nc.scalar.activation(qks[:], qkp[:], AF.Copy)
            vdp = pvd.tile([128, 2, 256], F32, tag="vd")
            for wi in range(2):
                for kc in range(2):
                    nc.tensor.matmul(
                        vdp[:, wi, :],
                        lhsT=xnt2[:, kc, wi, :], rhs=wv_s[kc][:],
                        start=(kc == 0), stop=(kc == 1),
                        skip_group_check=True,
                    )
            vs2 = pvs.tile([128, 2, 256], F16, tag="vs")
            nc.vector.tensor_copy(vs2[:], vdp[:])
            return qks, vs2

        # Software pipeline: while pair p's attention phases run, pair p+1's
        # projections and pair p+2's loads/transposes fill the PE stream, so
        # every consumer's input was produced a phase (or a full pair) ago.
        xn2_next = emit_loads(0)            # xn(0)
        xnt_next = emit_tr(xn2_next)        # xnt(0)
        xn2_next = emit_loads(1)            # xn(1)
        cur = emit_qkv(xnt_next)            # qks/vs(0)
        xnt_next = emit_tr(xn2_next)        # xnt(1)

        for p in range(npair):
            w0 = 2 * p
            qks, vs2 = cur

            # ---- loads for pair p+2 (DMA runs ahead) ----
            if p + 2 < npair:
                xn2_next_ = emit_loads(p + 2)
            else:
                xn2_next_ = None

            # ---- S^T row-tiled, hh-major: banks 2,3 (which carried the
            # previous pair's O/Wo outputs) come last ----
            sp = psp.tile([128, 4, 4, 128], F32, tag="sp")
            for hh in (0, 1, 2, 3):
                for mh in range(2):
                    for wi in range(2):
                        nc.tensor.matmul(
                            sp[:, hh, mh * 2 + wi, :],
                            lhsT=qks[32 * hh:32 * hh + 32, 2 + mh, wi * 128:(wi + 1) * 128],
                            rhs=qks[32 * hh:32 * hh + 32, mh, wi * 128:(wi + 1) * 128],
                            start=True, stop=True,
                            tile_position=(32 * hh, 0),
                            skip_group_check=True,
                        )
            # E^T = exp(S^T/sqrt(hd)); split by mh so D r=0 can start while
            # the second half still runs on ACT.
            es2 = pes.tile([128, 4, 2, 256], F16, tag="es")
            for mh in range(2):
                nc.scalar.activation(
                    es2[:, :, mh, :], sp[:, :, mh * 2:mh * 2 + 2, :],
                    AF.Exp, scale=inv_sqrt_hd)

            # ---- pair p+1 projections: PE work covering exp latency ----
            if p + 1 < npair:
                cur = emit_qkv(xnt_next)

            # ---- denominators D, pair-batched N=256 ----
            dp = pvd.tile([128, 2, 256], F32, tag="vd")
            for h in range(HEADS):
                r, j = h // 4, h % 4
                nc.tensor.matmul(
                    dp[32 * j:32 * j + 32, r, :],
                    lhsT=ones_s[:], rhs=es2[:, j, r, :],
                    start=(r == 0), stop=(r == 1), tile_position=(0, 32 * j),
                    skip_group_check=True,
                )
            dbs = pdbs.tile([128, 2, 256], F32, tag="dbs")  # (r, wi*128+l)
            for wi in range(2):
                nc.vector.reciprocal_approx_fast(
                    out=dbs[:, :, wi * 128:(wi + 1) * 128],
                    in_=dp[:, :, wi * 128:(wi + 1) * 128])

            # ---- transposes for pair p+2 (covers the dbs chain) ----
            if xn2_next_ is not None:
                xnt_next = emit_tr(xn2_next_)

            # ---- attention output + out-proj; op -> bank2, zp -> bank3 ----
            sp2 = psp.tile([128, 4, 4, 128], F32, tag="sp")
            os2 = pos.tile([128, 2, 2, 128], F16, tag="os")
            for wi in range(2):
                for h in range(HEADS):
                    r, j = h // 4, h % 4
                    nc.tensor.matmul(
                        sp2[32 * j:32 * j + 32, 2, 2 * wi + r, :],
                        lhsT=vs2[:, wi, 32 * h:32 * h + 32],
                        rhs=es2[:, j, r, wi * 128:(wi + 1) * 128],
                        start=(r == 0), stop=(r == 1), tile_position=(0, 32 * j),
                        skip_group_check=True,
                    )
                # fused normalize: os2[p, wi, r, l] = op * 1/D
                nc.vector.scalar_tensor_tensor(
                    out=os2[:, wi, :, :],
                    in0=sp2[:, 2, 2 * wi:2 * wi + 2, :],
                    scalar=1.0, in1=dbs[:, :, wi * 128:(wi + 1) * 128],
                    op0=ALU.mult, op1=ALU.mult,
                )
            for wi in range(2):
                for coh in range(2):
                    for kc in range(2):
                        nc.tensor.matmul(
                            sp2[:, 3, 2 * wi + coh, :],
                            lhsT=wo_s[kc][:, coh * 128:(coh + 1) * 128],
                            rhs=os2[:, wi, kc, :],
                            start=(coh == 0 and kc == 0),
                            stop=(coh == 1 and kc == 1),
                            skip_group_check=True,
                        )
            # one fp16 evac for both windows, then one store per window.
            zs = pzs.tile([128, 2, 2, 128], F16, tag="zs")  # (wi, coh, l)
            nc.scalar.activation(zs[:], sp2[:, 3, :, :], AF.Copy)
            for wi in range(2):
                w = w0 + wi
                nc.sync.dma_start(zt[w * 128:(w + 1) * 128, :], zs[:, wi, :, :])
    nc.compile()
    return nc


@functools.lru_cache(maxsize=2)
def _get_program(nw: int):
    return _build_program(nw)


def _im2win(x: np.ndarray) -> np.ndarray:
    """[B,T,H,W,C] -> [882,128,256] windows, flat order f = i_n*B + i_b."""
    s = x.strides
    xs = np.lib.stride_tricks.as_strided(
        x,
        shape=(B, T, NHW, PATCH, NHW, PATCH, C),
        strides=(s[0], s[1], STEP * s[2], s[2], STEP * s[3], s[3], s[4]),
    )
    w = xs.transpose(2, 4, 0, 1, 3, 5, 6)  # [iH,iW,b,t,p,q,c]
    return np.ascontiguousarray(w.reshape(NHW * NHW * B, L, C))


def _overlap_add(zwin: np.ndarray, bo: np.ndarray) -> np.ndarray:
    """[882,128,256] window outputs -> [B,T,H,W,C] with count-normalize + bo.

    Reproduces the reference's (n*b)->(b,n) flat-order reinterpretation.
    """
    th = np.arange(NHW) * STEP
    z = zwin.reshape(B, NHW, NHW, T, PATCH, PATCH, MID)  # [b,iH,iW,t,p,q,c]
    acc = np.zeros((B, T, H, W, MID), np.float32)
    count = np.zeros((H, W), np.float32)
    for p in range(PATCH):
        rid = (th + p)[:, None]
        for q in range(PATCH):
            cid = (th + q)[None, :]
            acc[:, :, rid, cid, :] += z[:, :, :, :, p, q, :].transpose(0, 3, 1, 2, 4)
            count[rid, cid] += 1.0
    out = acc / count[None, None, :, :, None] + bo[None, None, None, None, :]
    return out


LAST_RESULT = None


def kernel(x, ln_g, ln_b, Wq, Wk, Wv, Wo, bo):
    x = np.asarray(x, np.float32)
    ln_g = np.asarray(ln_g, np.float32)
    ln_b = np.asarray(ln_b, np.float32)
    assert np.allclose(ln_b, 0.0), "kernel folds ln_g into weights; ln_b must be 0"
    # Fold LN gamma into the input side of Wq/Wk/Wv.
    wq_t = np.ascontiguousarray((np.asarray(Wq, np.float32) * ln_g).T.astype(np.float16))
    wk_t = np.ascontiguousarray((np.asarray(Wk, np.float32) * ln_g).T.astype(np.float16))
    wv_t = np.ascontiguousarray((np.asarray(Wv, np.float32) * ln_g).T.astype(np.float16))
    wo_t = np.ascontiguousarray(np.asarray(Wo, np.float32).T.astype(np.float16))
    ones1 = np.ones((128, 32), np.float16)
    ident = np.eye(128, dtype=np.float16)

    # LayerNorm on the host (pure data prep; gamma folded into W, beta=0),
    # then im2win.  fp16 halves the per-window DMA load.
    mu = x.mean(-1, keepdims=True)
    var = x.var(-1, keepdims=True)
    xnh = ((x - mu) / np.sqrt(var + EPS)).astype(np.float16)
    win = _im2win(xnh)                            # [882, 128, 256] fp16
    pad = NCORES * NW - NWIN                      # 14
    winp = np.concatenate([win, np.zeros((pad, L, C), np.float16)], 0)
    shards = winp.reshape(NCORES, NW * L, C)

    nc = _get_program(NW)
    trace = bool(int(__import__("os").environ.get("KERNEL_TRACE", "0")))
    in_maps = []
    for i in range(NCORES):
        in_maps.append({
            "xw": np.ascontiguousarray(shards[i]),
            "wq": wq_t, "wk": wk_t, "wv": wv_t, "wo": wo_t,
            "ones1": ones1, "ident": ident,
        })
    res = run_bass_kernel_spmd(nc, in_maps, core_ids=list(range(NCORES)),
                               trace=trace)
    global LAST_RESULT
    LAST_RESULT = res
    # zt rows w*128+p, cols coh*128+l hold Z_w[l, coh*128+p]
    zts = [res.results[i]["zt"].reshape(NW, 128, 2, 128) for i in range(NCORES)]
    zall = np.concatenate(zts, 0).astype(np.float32)   # [896, p, coh, l]
    zwin = zall.transpose(0, 3, 2, 1).reshape(NCORES * NW, L, MID)[:NWIN]
    return _overlap_add(np.ascontiguousarray(zwin), np.asarray(bo, np.float32))
